# revision 3
# baseline (speedup 1.0000x reference)
# Trainium2 Bass kernel for nn_ContinuousHopfieldNet_70652212019686.
#
# Math used (verified numerically against the jax reference):
#   - The rectangular basis makes A = F@F.T + 0.5*I exactly diagonal 4.5*I:
#     every padded position falls strictly inside one bin (margin ~1.2e-4 vs
#     fp32 rounding ~2e-7), 4 positions per bin.  Hence
#         G[p, i] = (p//4 == i) / 4.5   and   B = G.T @ k  is a binsum:
#         B[i, :] = (k[4i] + k[4i+1] + k[4i+2] + k[4i+3]) / 4.5
#   - keys(t) sampled on the 2048-point grid is a gather of B rows
#     (every grid point lies in exactly one bin, except t=1.0 which lies in
#     none -> zero scores column).  The whole 2048-point softmax collapses to
#     1024 bins with aggregated trapezoid weights wbin[i], plus a correction
#     term w_none * exp(-m) in the normalizer for the t=1.0 point.
#   - Each retrieval iteration is then:
#         S  = q @ B.T                        (nq x nb)
#         m  = max(rowmax(S), 0)
#         E  = exp(S - m)
#         Z  = E @ wbin + w_none * exp(-m)
#         q' = (E @ (wbin[:,None] * B)) / Z   (nq x d)
#
# Sharding: queries (nq=1024) split across 8 cores, 128 per core.  B machinery
# is recomputed per-core from the full k (replicated); iteration-1's S matmul
# accumulates per bin-chunk so it pipelines under the k DMA.
#
# Matmul precision ("bf16s" mode): X@Y with X=Xh+Xl, Y=Yh+Yl split into bf16
# halves; Xh@Yh + Xh@Yl + Xl@Yh at 1 cyc/row beats fp32's 4 cyc/row by 25%
# while keeping ~2^-18 effective operand precision (the retrieval argmax is
# basin-sensitive: plain bf16/fp32r flip retrievals; split verified ~5e-6).
import os

import numpy as np

NB = 1024
D = 1024
KLEN = 4096
NQ = 1024
NPTS = 2048
NCORES = 8
QS = NQ // NCORES
NITER = 3

# "f32" = plain fp32 matmuls; "bf16s" = 3-term split-bf16 matmuls
MM_DTYPE = os.environ.get("KERNEL_MM_DTYPE", "bf16s")


def _host_constants():
    """Input-independent basis constants, replicating reference fp32 math.

    Bin edges are exact multiples of 1/1024 in fp32, so bin membership of the
    fp32 grid values t_j is computed exactly.  Verified bit-identical to the
    jax reference in test.py.
    """
    t = np.linspace(0.0, 1.0, NPTS).astype(np.float32)  # correctly-rounded j/2047
    dt = np.diff(t)
    w = np.concatenate([dt[:1] / 2, (dt[:-1] + dt[1:]) / 2, dt[-1:] / 2]).astype(
        np.float32
    )
    edges = (np.arange(NB + 1, dtype=np.float64) / NB).astype(np.float32)
    lb, ub = edges[:-1], edges[1:]
    cand = np.clip(np.searchsorted(ub, t, side="right"), 0, NB - 1)
    ok = (t >= lb[cand]) & (t < ub[cand])
    wbin64 = np.zeros(NB)
    np.add.at(wbin64, cand[ok], w[ok].astype(np.float64))
    wbin = wbin64.astype(np.float32)
    w_none = float(w[~ok].astype(np.float64).sum())
    # [128, 8] per-(partition, bin-chunk) layouts for the device
    wzc = wbin.reshape(8, 128).T.copy()  # wzc[p, c] = wbin[128c + p]
    wdiv = (wzc * np.float32(1.0 / 4.5)).astype(np.float32)
    # Z-matmul rhs padded to N=2 (even innermost free counts keep all modes legal)
    wz = np.zeros((128, 8, 2), np.float32)
    wz[:, :, 0] = wzc
    return wz, wdiv, w_none


def _build_program(bench_trips=0, bench_scope="iters"):
    """bench_trips>0 wraps part of the body in an on-device For_i loop so
    steady-state per-body time can be extracted from wall-clock deltas (no
    NTFF profiling is available under this axon client).  bench_scope:
    "iters" loops only the 3 retrieval iterations; "full" loops everything
    including the k DMA + B build."""
    import concourse.bacc as bacc
    import concourse.tile as tile
    from concourse import mybir
    from concourse.masks import make_identity

    F32 = mybir.dt.float32
    BF16 = mybir.dt.bfloat16
    split = MM_DTYPE == "bf16s"
    MMDT = BF16 if split else F32

    _, _, w_none = _host_constants()
    ln_wnone = float(np.log(np.float64(w_none)))

    nc = bacc.Bacc(
        "TRN2",
        target_bir_lowering=False,
        debug=False,
        enable_asserts=True,
        num_devices=NCORES,
    )
    kk = nc.dram_tensor("kk", [KLEN, D], F32, kind="ExternalInput").ap()
    qs = nc.dram_tensor("qs", [QS, D], F32, kind="ExternalInput").ap()
    wz_d = nc.dram_tensor("wz", [128, 8, 2], F32, kind="ExternalInput").ap()
    wdiv_d = nc.dram_tensor("wdiv", [128, 8], F32, kind="ExternalInput").ap()
    out_d = nc.dram_tensor("out", [QS, D], F32, kind="ExternalOutput").ap()

    with tile.TileContext(nc) as tc:
        with (
            tc.tile_pool(name="const", bufs=1) as constp,
            tc.tile_pool(name="ksrc", bufs=2) as kpool,
            tc.tile_pool(name="work", bufs=2) as work,
            tc.tile_pool(name="iterp", bufs=2) as iterp,
            tc.tile_pool(name="stats", bufs=4) as stats,
            tc.tile_pool(name="psS", bufs=2, space="PSUM") as psS,
            tc.tile_pool(name="psT", bufs=3, space="PSUM") as psT,
            tc.tile_pool(name="psZ", bufs=1, space="PSUM") as psZ,
        ):
            ident = constp.tile([128, 128], F32)
            make_identity(nc, ident)
            lnw_sb = constp.tile([128, 1], F32)
            nc.vector.memset(lnw_sb, ln_wnone)
            wz_sb = constp.tile([128, 8, 2], F32)
            nc.sync.dma_start(wz_sb, wz_d)
            wdiv_sb = constp.tile([128, 8], F32)
            nc.sync.dma_start(wdiv_sb, wdiv_d)
            # Z-matmul weights in matmul dtype (hi/lo pair when splitting)
            wz_hi = constp.tile([128, 8, 2], MMDT)
            nc.vector.tensor_copy(wz_hi, wz_sb)
            wz_lo = None
            if split:
                wz_lo = constp.tile([128, 8, 2], MMDT)
                nc.vector.tensor_tensor(
                    wz_lo, wz_sb, wz_hi, mybir.AluOpType.subtract
                )

            def copy_ps(dst, src, i):
                if i % 2 == 0:
                    nc.vector.tensor_copy(dst, src)
                else:
                    nc.scalar.copy(dst, src)

            def split_from(dst_hi, dst_lo, src_f32, i):
                """dst_hi = bf16(src); dst_lo = bf16(src - dst_hi)."""
                copy_ps(dst_hi, src_f32, i)
                if dst_lo is not None:
                    nc.vector.tensor_tensor(
                        dst_lo, src_f32, dst_hi, mybir.AluOpType.subtract
                    )

            def trans_to(dsts, src_2d, nblk):
                """dsts = (hi, lo_or_None); hi[:, i, :] = block-i transpose of
                src_2d.  4 transposes share one PSUM tile, then one merged
                copy (plus one subtract when splitting)."""
                hi, lo = dsts
                assert nblk % 4 == 0
                for h in range(nblk // 4):
                    pt4 = psT.tile([128, 512], F32, tag="pt4")
                    for j in range(4):
                        nc.tensor.transpose(
                            pt4[:, 128 * j : 128 * (j + 1)],
                            src_2d[:, 128 * (4 * h + j) : 128 * (4 * h + j + 1)],
                            ident,
                        )
                    pv = pt4.rearrange("p (a b) -> p a b", a=4)
                    split_from(
                        hi[:, 4 * h : 4 * h + 4, :],
                        None if lo is None else lo[:, 4 * h : 4 * h + 4, :],
                        pv,
                        h,
                    )

            def terms(xpair, ypair):
                """Accumulation terms: 1 for f32, 3 for split-bf16."""
                xh, xl = xpair
                yh, yl = ypair
                if not split:
                    return [(xh, yh)]
                return [(xh, yh), (xh, yl), (xl, yh)]

            def mm_accum(out_ap, lhs_pair_fn, rhs_pair_fn, nk, max_terms=3):
                """out = sum_k lhsT_k.T @ rhs_k with split terms.  max_terms=2
                drops the xl@yh term — safe only where the lhs is near
                bf16-exact (the final U: P3 is ~one-hot; verified 4.9e-4
                absmax on the grading data in mc_precision.py)."""
                first = True
                for kd in range(nk):
                    tt = terms(lhs_pair_fn(kd), rhs_pair_fn(kd))[:max_terms]
                    for ti, (lh, rh) in enumerate(tt):
                        last = kd == nk - 1 and ti == len(tt) - 1
                        nc.tensor.matmul(out_ap, lh, rh, start=first, stop=last)
                        first = False

            # B in both layouts (hi/lo bf16 pairs when splitting):
            #   Bw[p, c, d]  = wbin[b]/4.5 * Bsum[b, d]  (b = 128c + p) [bins on part]
            #   BT[p, kd, b] = B[b, 128*kd + p]                         [d on part]
            def build_b():
                Bw_hi = constp.tile([128, 8, D], MMDT, tag="Bw_hi")
                BT_hi = constp.tile([128, 8, NB], MMDT, tag="BT_hi")
                Bw_lo = BT_lo = None
                if split:
                    Bw_lo = constp.tile([128, 8, D], MMDT, tag="Bw_lo")
                    BT_lo = constp.tile([128, 8, NB], MMDT, tag="BT_lo")
                kk_r = kk.rearrange("(c p r) d -> c p r d", p=128, r=4)
                for c in range(8):
                    kt = kpool.tile([128, 4, D], F32, tag="kt")
                    nc.sync.dma_start(kt, kk_r[c])
                    bsum = work.tile([128, D], F32, tag="bsum")
                    nc.vector.tensor_add(bsum, kt[:, 0], kt[:, 1])
                    nc.vector.tensor_add(bsum, bsum, kt[:, 2])
                    nc.vector.tensor_add(bsum, bsum, kt[:, 3])
                    if split:
                        bwf = work.tile([128, D], F32, tag="bwf")
                        nc.vector.tensor_scalar_mul(bwf, bsum, wdiv_sb[:, c : c + 1])
                        split_from(Bw_hi[:, c], Bw_lo[:, c], bwf, 0)
                    else:
                        nc.vector.tensor_scalar_mul(
                            Bw_hi[:, c], bsum, wdiv_sb[:, c : c + 1]
                        )
                    bplain = work.tile([128, D], F32, tag="bplain")
                    nc.scalar.mul(bplain, bsum, 1.0 / 4.5)
                    for h in range(2):
                        pt4 = psT.tile([128, 512], F32, tag="pt4")
                        for j in range(4):
                            kd = 4 * h + j
                            nc.tensor.transpose(
                                pt4[:, 128 * j : 128 * (j + 1)],
                                bplain[:, 128 * kd : 128 * (kd + 1)],
                                ident,
                            )
                        pv = pt4.rearrange("p (a b) -> p a b", a=4)
                        split_from(
                            BT_hi[:, 4 * h : 4 * h + 4, 128 * c : 128 * (c + 1)],
                            None
                            if BT_lo is None
                            else BT_lo[:, 4 * h : 4 * h + 4, 128 * c : 128 * (c + 1)],
                            pv,
                            h,
                        )
                return (Bw_hi, Bw_lo), (BT_hi, BT_lo)

            def build_q0():
                # initial transposed queries Qt[p, kd, j] = q[j, 128*kd + p]
                qn = work.tile([128, D], F32, tag="qn")
                nc.sync.dma_start(qn, qs)
                Qt_hi = iterp.tile([128, 8, QS], MMDT, tag="qt_hi")
                Qt_lo = iterp.tile([128, 8, QS], MMDT, tag="qt_lo", name="qt_lo") if split else None
                trans_to((Qt_hi, Qt_lo), qn, 8)
                return (Qt_hi, Qt_lo)

            def iterations(Bw, BT, Qt):
                Bw_hi, Bw_lo = Bw
                BT_hi, BT_lo = BT
                for it in range(NITER):

                    def qt_pair(kd, Qt=Qt):
                        return (
                            Qt[0][:, kd],
                            None if Qt[1] is None else Qt[1][:, kd],
                        )

                    # S = q @ B.T : lhsT = Qt chunks (d on part), rhs = BT
                    S = psS.tile([128, NB], F32, tag="S")
                    if it == 0:
                        # bin-chunk accumulation: S[:, 128c:...] needs only
                        # k-chunk c, so iter-1 S pipelines with the k DMA
                        for c in range(8):
                            cs = slice(128 * c, 128 * (c + 1))
                            mm_accum(
                                S[:, cs],
                                qt_pair,
                                lambda kd, cs=cs: (
                                    BT_hi[:, kd, cs],
                                    None if BT_lo is None else BT_lo[:, kd, cs],
                                ),
                                8,
                            )
                    else:
                        for n in range(2):
                            ns = slice(512 * n, 512 * (n + 1))
                            mm_accum(
                                S[:, ns],
                                qt_pair,
                                lambda kd, ns=ns: (
                                    BT_hi[:, kd, ns],
                                    None if BT_lo is None else BT_lo[:, kd, ns],
                                ),
                                8,
                            )
                    nm = stats.tile([128, 1], F32, tag="nm")
                    nc.vector.reduce_max(
                        nm, S, axis=mybir.AxisListType.X, negate=True
                    )
                    negm = stats.tile([128, 1], F32, tag="negm")
                    nc.vector.tensor_scalar_min(negm, nm, 0.0)
                    E = iterp.tile([128, NB], F32, tag="E")
                    nc.scalar.activation(
                        E, S, mybir.ActivationFunctionType.Exp, bias=negm, scale=1.0
                    )
                    # E transposed for contraction over bins
                    ET_hi = iterp.tile([128, 8, QS], MMDT, tag="et_hi")
                    ET_lo = (
                        iterp.tile([128, 8, QS], MMDT, tag="et_lo", name="et_lo")
                        if split
                        else None
                    )
                    trans_to((ET_hi, ET_lo), E, 8)

                    def et_pair(c):
                        return (
                            ET_hi[:, c],
                            None if ET_lo is None else ET_lo[:, c],
                        )

                    # Z = E @ wbin (N=2, zero-padded)
                    Zp = psZ.tile([128, 2], F32, tag="Z")
                    mm_accum(
                        Zp,
                        et_pair,
                        lambda c: (
                            wz_hi[:, c],
                            None if wz_lo is None else wz_lo[:, c],
                        ),
                        8,
                    )
                    # U = E @ Bw; final iteration drops the El@Bwh term (the
                    # retrieval is ~one-hot by iter 3, E is near bf16-exact)
                    U = psS.tile([128, D], F32, tag="S")
                    for n in range(2):
                        ns = slice(512 * n, 512 * (n + 1))
                        mm_accum(
                            U[:, ns],
                            et_pair,
                            lambda c, ns=ns: (
                                Bw_hi[:, c, ns],
                                None if Bw_lo is None else Bw_lo[:, c, ns],
                            ),
                            8,
                            max_terms=(2 if it == NITER - 1 else 3),
                        )
                    # zf = Z + w_none * exp(-m);  recip = 1/zf
                    zc = stats.tile([128, 1], F32, tag="zc")
                    nc.scalar.activation(
                        zc,
                        negm,
                        mybir.ActivationFunctionType.Exp,
                        scale=1.0,
                        bias=lnw_sb[:, :1],
                    )
                    zf = stats.tile([128, 1], F32, tag="zf")
                    nc.vector.tensor_add(zf, Zp[:, 0:1], zc)
                    rc = stats.tile([128, 1], F32, tag="rc")
                    nc.vector.reciprocal(rc, zf)
                    # q' = U / Z
                    Un = iterp.tile([128, D], F32, tag="un")
                    nc.scalar.mul(Un, U, rc)
                    if it < NITER - 1:
                        Qt_hi = iterp.tile([128, 8, QS], MMDT, tag="qt_hi")
                        Qt_lo = (
                            iterp.tile([128, 8, QS], MMDT, tag="qt_lo", name="qt_lo")
                            if split
                            else None
                        )
                        trans_to((Qt_hi, Qt_lo), Un, 8)
                        Qt = (Qt_hi, Qt_lo)
                    else:
                        nc.sync.dma_start(out_d, Un)

            if bench_trips and bench_scope == "full":
                with tc.For_i(0, bench_trips, 1):
                    Qt = build_q0()
                    Bw, BT = build_b()
                    iterations(Bw, BT, Qt)
            elif bench_trips:
                Qt = build_q0()
                Bw, BT = build_b()
                with tc.For_i(0, bench_trips, 1):
                    iterations(Bw, BT, Qt)
            else:
                Qt = build_q0()
                Bw, BT = build_b()
                iterations(Bw, BT, Qt)

    nc.compile()
    return nc


_CACHE = {}
LAST_RESULTS = None


def kernel(**inputs):
    global LAST_RESULTS
    k = np.ascontiguousarray(np.asarray(inputs["k"], dtype=np.float32))
    q = np.ascontiguousarray(np.asarray(inputs["q"], dtype=np.float32))
    assert k.shape == (KLEN, D) and q.shape == (NQ, D)

    if "nc" not in _CACHE:
        _CACHE["nc"] = _build_program()
        _CACHE["consts"] = _host_constants()
    nc = _CACHE["nc"]
    wz, wdiv, _ = _CACHE["consts"]

    in_maps = []
    for c in range(NCORES):
        in_maps.append(
            {
                "kk": k,
                "qs": np.ascontiguousarray(q[QS * c : QS * (c + 1)]),
                "wz": wz,
                "wdiv": wdiv,
            }
        )

    import concourse.bass_utils as bass_utils

    res = bass_utils.run_bass_kernel_spmd(
        nc, in_maps, core_ids=list(range(NCORES))
    )
    LAST_RESULTS = res
    out = np.concatenate([res.results[c]["out"] for c in range(NCORES)], axis=0)
    return np.ascontiguousarray(out, dtype=np.float32)


if __name__ == "__main__":
    rng = np.random.default_rng(0)
    k = rng.standard_normal((KLEN, D), dtype=np.float32)
    q = rng.standard_normal((NQ, D), dtype=np.float32)
    o = kernel(k=k, q=q)
    print("kernel ran, out shape", o.shape, "finite:", np.isfinite(o).all())

